# revision 1
# baseline (speedup 1.0000x reference)
"""Trainium2 Bass kernel for LongRangeTCN (4-layer dilated causal conv + BN + LIF + residual).

Sharding: data-parallel over batch B=32 -> 4 per core across 8 NeuronCores.
Per core layout (SBUF, fp32):
  X   [128, 4, 4112]  residual/input; cols [0,16) zero pad (conv halo), col 16+t = x_t
  XH  [128, 4, 4224]  scan input/trajectory; cols [0,64) zero (warmup), col 64+t holds
                      xh_t = 0.5*BN(conv(x))_t, overwritten in-place by A_t during the scan
  WT  [128, 4, 3, 128] folded conv weights (lhsT per layer/tap)
  BIAS [128, 4]        folded BN bias (per-channel) * 0.5

Per layer: conv = 3 shifted matmuls accumulated in PSUM (tap k reads x at t-(2-k)*d),
ACT evacuates PSUM->SBUF adding bias. LIF scan: v' = (A<1)*A with A = 0.5*v + xh_t,
run as 32 parallel chunks of 128 steps per batch with a 64-step warmup (0.5 decay/step
makes chunk-boundary state influence vanish below fp32 noise). Spikes s=(A>=1) and the
residual add are one full-width fused op at the end of each layer.
"""

import numpy as np

TAU, VTH, EPS, K = 2.0, 1.0, 1e-5, 3
DILATIONS = (1, 2, 4, 8)
B, C, T = 32, 128, 4096
NCORES = 8
BL = B // NCORES          # 4 batches per core
H = 64                    # scan warmup steps
LC = 128                  # scan chunk length
NCH = T // LC             # 32 chunks per batch
PADX = 16                 # conv left halo (max (K-1)*d = 16)
SX = PADX + T             # 4112
SXH = H + T + (LC - (H + T) % LC) % LC  # pad up to multiple of LC: 4224
NCH_XH = SXH // LC        # 33

_cache = {}


def _build():
    import concourse.bass as bass
    import concourse.bacc as bacc
    import concourse.tile as tile
    import concourse.mybir as mybir

    dt = mybir.dt.float32
    Alu = mybir.AluOpType
    Act = mybir.ActivationFunctionType

    nc = bacc.Bacc("TRN2", target_bir_lowering=False, debug=False)
    x_d = nc.dram_tensor("x", [BL, C, T], dt, kind="ExternalInput")
    wt_d = nc.dram_tensor("wt", [C, 4, K, C], dt, kind="ExternalInput")
    b_d = nc.dram_tensor("bias", [C, 4], dt, kind="ExternalInput")
    o_d = nc.dram_tensor("out", [BL, C, T], dt, kind="ExternalOutput")

    with tile.TileContext(nc) as tc:
        with (
            tc.tile_pool(name="big", bufs=1) as big,
            tc.tile_pool(name="small", bufs=1) as small,
            tc.tile_pool(name="psum", bufs=4, space="PSUM") as pp,
        ):
            X = big.tile([C, BL, SX], dt, tag="X")
            XH = big.tile([C, BL, SXH], dt, tag="XH")
            WT = small.tile([C, 4, K, C], dt, tag="WT")
            BIAS = small.tile([C, 4], dt, tag="BIAS")
            V = small.tile([C, BL, NCH], dt, tag="V")
            SCR = small.tile([C, BL, NCH], dt, tag="SCR")

            nc.sync.dma_start(WT[:], wt_d[:])
            nc.sync.dma_start(BIAS[:], b_d[:])
            nc.vector.memset(X[:, :, 0:PADX], 0.0)
            nc.vector.memset(XH[:, :, 0:H], 0.0)
            for b in range(BL):
                nc.sync.dma_start(X[:, b, PADX:SX], x_d[b])

            XH4 = XH[:].rearrange("p a (c l) -> p a c l", l=LC)

            for li, d in enumerate(DILATIONS):
                # conv + BN-bias -> xh
                for b in range(BL):
                    for t0 in range(0, T, 512):
                        ps = pp.tile([C, 512], dt, tag="ps")
                        for k in range(K):
                            sh = (K - 1 - k) * d
                            nc.tensor.matmul(
                                ps[:],
                                WT[:, li, k, :],
                                X[:, b, PADX + t0 - sh : PADX + t0 - sh + 512],
                                start=(k == 0),
                                stop=(k == K - 1),
                            )
                        nc.scalar.activation(
                            XH[:, b, H + t0 : H + t0 + 512], ps[:],
                            Act.Identity, bias=BIAS[:, li : li + 1], scale=1.0,
                        )
                # LIF scan: chunks in parallel, sequential over H+LC steps
                nc.vector.memset(V[:], 0.0)
                for j in range(H + LC):
                    if j < LC:
                        col = XH4[:, :, 0:NCH, j]
                    else:
                        col = XH4[:, :, 1 : NCH + 1, j - LC]
                    a_dst = SCR[:] if j < H else col
                    # A = 0.5*v + xh_t   (overwrites xh col in place when j>=H)
                    nc.vector.scalar_tensor_tensor(
                        a_dst, V[:], 0.5, col, op0=Alu.mult, op1=Alu.add
                    )
                    # v' = (A < 1) * A
                    nc.vector.scalar_tensor_tensor(
                        V[:], a_dst, float(VTH), a_dst, op0=Alu.is_lt, op1=Alu.mult
                    )
                # x += (A >= 1)  (spikes + residual, one fused op)
                nc.vector.scalar_tensor_tensor(
                    X[:, :, PADX:SX], XH[:, :, H : H + T], float(VTH),
                    X[:, :, PADX:SX], op0=Alu.is_ge, op1=Alu.add,
                )

            for b in range(BL):
                nc.sync.dma_start(o_d[b], X[:, b, PADX:SX])

    nc.compile()
    return nc


def kernel(x, w, gamma, beta, mean, var, **_):
    from concourse.bass_utils import run_bass_kernel_spmd

    x = np.ascontiguousarray(x, np.float32)
    inv = (gamma / np.sqrt(var + EPS)).astype(np.float32)          # [4, C]
    # wt[ci, l, k, co] = 0.5 * w[l, co, ci, k] * inv[l, co]
    wt = (0.5 * w * inv[:, :, None, None]).astype(np.float32)      # [4, Co, Ci, K]
    wt = np.ascontiguousarray(wt.transpose(2, 0, 3, 1))            # [Ci, 4, K, Co]
    bias = (0.5 * (beta - mean * inv)).astype(np.float32).T        # [C, 4]
    bias = np.ascontiguousarray(bias)

    if "nc" not in _cache:
        _cache["nc"] = _build()
    nc = _cache["nc"]

    in_maps = [
        {"x": np.ascontiguousarray(x[i * BL : (i + 1) * BL]), "wt": wt, "bias": bias}
        for i in range(NCORES)
    ]
    res = run_bass_kernel_spmd(nc, in_maps, list(range(NCORES)))
    return np.concatenate([res.results[i]["out"] for i in range(NCORES)], axis=0)
